# revision 25
# baseline (speedup 1.0000x reference)
"""Trainium2 Bass kernel for nn_MultiHeadHCGAttention.

Math notes (exact restructuring of the reference):
  The key_padding_mask replaces the ENTIRE key feature row with -1e9 BEFORE
  the K projection (v is NOT masked). Hence every masked key position s in
  batch b has the SAME projected K row:
      Kmask[n] = -1e9 * sum_h Wk[n,h,:] + bk[n]   (data independent)
  All masked keys share one score z = Q.Kmask/sqrt(dk) with |z| ~ 1e9.
  In fp32 softmax the output per (query q, head n) is therefore either
    - mean of V over the masked key positions  if z > max unmasked score
      (uniform softmax over the identical-score masked keys)
    - standard softmax over unmasked keys      otherwise (masked weights
      underflow to exactly 0 in fp32)
  The boundary band has probability ~1e-7 per query -> decided by sign(z),
  computed exactly on the host in fp64 (z = q @ (Wq@Kmask) + bq.Kmask).

  Device computes bf16 attention over the gathered unmasked keys only
  (normal O(1) magnitudes); rows whose head chose the mask branch get
  denom += 1e30 on device (output underflows to 0) and the contribution
  ubar[b,n] = (mean_masked V[b,n]) @ Wo_n is added on the host in fp64.
  bv is folded into bo on the host (softmax weights sum to 1 so
  out = PV/d + bv exactly before the output projection).

Sharding: 8 cores = (batch b in 0..3) x (query half). No collectives (the
pairwise AllGather fabric here moves ~38GB/s -- too slow to pay for
de-duplicating the K/V projections).

Schedule: projections and attention are interleaved so the Scalar engine's
exp stream (the attention-phase bottleneck) overlaps projection matmuls,
and attention runs qc-major with the first query-chunk's output projection
emitted piecewise between second-chunk heads. The softmax denominator is a
DVE bf16 chain over the exp tiles + one 512-cycle matmul per chunk
(deferred one chunk so the PE never waits on the chain).
"""

import math
import sys

if "/opt/trn_rl_repo" not in sys.path:
    sys.path.insert(0, "/opt/trn_rl_repo")

import ml_dtypes
import numpy as np

import concourse.bacc as bacc
import concourse.tile as tile
from concourse import mybir
from concourse.bass_utils import run_bass_kernel_spmd

S, B, H = 2048, 4, 1024
NH, DK = 8, 128
NHDK = NH * DK
NEG = -1.0e9
NCORES = 8
HT = H // 128  # 8 H-tiles

bf16 = mybir.dt.bfloat16
f32 = mybir.dt.float32
npbf16 = ml_dtypes.bfloat16

_PROG_CACHE: dict = {}


def build_program(Sq: int, UP: int):
    """Emit the per-core SPMD program. Sq = queries per core, UP =
    unmasked-key count (max over batches)."""
    NKT = (UP + 127) // 128
    ktiles = [(o, min(128, UP - o)) for o in range(0, UP, 128)]
    QC = Sq // 512  # 512-wide query chunks
    # key free-dim chunks for the K projection. First chunk is narrow (256)
    # so the critical-path DMA (kuT_first + wk[0]) is small and the first
    # matmul chain starts ASAP. kuT is shipped key-chunk-major (all H-tiles
    # of one chunk contiguous per partition -> 4-8KB DMA lines, ~3x the ring
    # bandwidth of the 512B lines a plain [H, UP] slice would give).
    K0 = min(256, UP)
    kchunks = [(0, K0)]
    o = K0
    while o < UP:
        w = min(512, UP - o)
        kchunks.append((o, w))
        o += w
    NKR = len(kchunks) - 1  # 512-wide rest chunks (last one ragged)

    nc = bacc.Bacc("TRN2", target_bir_lowering=False, debug=False)

    d_qT = nc.dram_tensor("qT", [H, Sq], bf16, kind="ExternalInput")
    d_kuF = nc.dram_tensor("kuF", [128, HT * K0], bf16, kind="ExternalInput")
    d_kuR = nc.dram_tensor(
        "kuR", [128, max(NKR, 1) * HT * 512], bf16, kind="ExternalInput"
    )
    d_vuT = nc.dram_tensor("vuT", [H, UP], bf16, kind="ExternalInput")
    d_wq = nc.dram_tensor("wq", [H, NHDK], bf16, kind="ExternalInput")
    d_wk = nc.dram_tensor("wk", [4 * 128, HT * 128], bf16, kind="ExternalInput")
    d_wv = nc.dram_tensor("wv", [H, 512], bf16, kind="ExternalInput")
    d_wo = nc.dram_tensor("wo", [NHDK, H], bf16, kind="ExternalInput")
    d_bq = nc.dram_tensor("bq", [DK, NH], f32, kind="ExternalInput")
    d_bk = nc.dram_tensor("bk", [DK, 4], f32, kind="ExternalInput")
    d_sel = nc.dram_tensor("sel", [128, 2], f32, kind="ExternalInput")
    d_bo = nc.dram_tensor("bo", [128, HT], f32, kind="ExternalInput")
    d_padb = nc.dram_tensor("padb", [128, NKT], f32, kind="ExternalInput")
    d_chb = nc.dram_tensor("chb", [1, NH * Sq], bf16, kind="ExternalInput")
    d_yT = nc.dram_tensor("yT", [H, Sq], f32, kind="ExternalOutput")

    SCALE = 1.0 / math.sqrt(DK)

    with tile.TileContext(nc) as tc:
        with (
            tc.tile_pool(name="const", bufs=1) as const,
            tc.tile_pool(name="exp", bufs=3) as expp,
            tc.tile_pool(name="es", bufs=3) as esp,
            tc.tile_pool(name="sc", bufs=2) as scp,
            tc.tile_pool(name="bc", bufs=3) as bcp,
            tc.tile_pool(name="yt", bufs=2) as ytp,
            tc.tile_pool(name="ps_proj", bufs=4, space="PSUM") as ps_proj,
            tc.tile_pool(name="ps_pv", bufs=3, space="PSUM") as ps_pv,
            tc.tile_pool(name="ps_d", bufs=1, space="PSUM") as ps_d,
            tc.tile_pool(name="dram", bufs=1, space="DRAM") as dram,
            tc.tile_pool(name="stg", bufs=2) as stgp,
        ):
            qT = const.tile([128, HT, Sq], bf16)
            kuF = const.tile([128, HT, K0], bf16)
            kuR = const.tile([128, max(NKR, 1), HT, 512], bf16)
            vuT = const.tile([128, HT, UP], bf16)
            wq = const.tile([128, HT, NHDK], bf16)
            wk = const.tile([128, 4, HT, 128], bf16)
            wv = const.tile([128, HT, 512], bf16)
            wo = const.tile([128, NH, H], bf16)
            bq = const.tile([128, NH], f32)
            bk = const.tile([128, 4], f32)
            bo = const.tile([128, HT], f32)
            padb = const.tile([128, NKT], f32)
            sel = const.tile([128, 2], f32)
            ones_mat = const.tile([128, 128], bf16)
            nc.vector.memset(ones_mat[:], 1.0)
            ksb = const.tile([128, NH, UP], bf16)
            vg = const.tile([128, 2, NKT, 512], bf16)
            qsb = const.tile([128, NH, Sq], bf16)
            out_all = const.tile([128, NH, Sq], bf16)
            # DRAM bounce + gather buffers for the pairwise K/V exchange
            kown = dram.tile([128, 4, UP], bf16)
            kall = dram.tile([2, 128, 4, UP], bf16)
            vown = dram.tile([128, NKT, 512], bf16)
            vall = dram.tile([2, 128, NKT, 512], bf16)

            r_qT = d_qT[:].rearrange("(t p) s -> p t s", p=128)
            r_kuF = d_kuF[:].rearrange("p (t w) -> p t w", t=HT)
            r_kuR = d_kuR[:].rearrange("p (c t w) -> p c t w", t=HT, w=512)
            r_vuT = d_vuT[:].rearrange("(t p) u -> p t u", p=128)
            r_wq = d_wq[:].rearrange("(t p) d -> p t d", p=128)
            r_wk = d_wk[:].rearrange("(n p) (t c) -> p n t c", p=128, t=HT)
            r_wv = d_wv[:].rearrange("(t p) d -> p t d", p=128)
            r_wo = d_wo[:].rearrange("(n p) h -> p n h", p=128)

            # HAM warmup: the PE clock-gate needs ~3.4us of sustained matmul
            # activity to go 1.2 -> 2.4 GHz. The first ~10us of the kernel
            # are DMA-bound with the PE idle, so spend them on dummy matmuls
            # (ones@ones into the rarely-used denominator PSUM bank) so the
            # real matmul stream starts warm.
            pwarm = ps_d.tile([128, 512], f32, tag="pd")
            for _ in range(48):
                nc.tensor.matmul(
                    pwarm[:, 0:64], ones_mat[:], ones_mat[:, 0:64],
                    start=True, stop=True,
                )

            # Input DMA on BOTH HWDGE rings (SP=nc.sync, ACT=nc.scalar), in
            # strict consumption order per ring. Critical path first: the
            # K0-wide first kproj chunk needs kuT[:, :, :K0] + wk[0] + bk
            # only (~0.8 MB split across the rings) instead of all of kuT.
            # The ACT ring is only used before the attention phase (its
            # queue runs the exp stream afterwards) and again for the
            # final output pieces (exp is done by then).
            # Ring assignment in strict per-ring consumption order. The kproj
            # stream (wk heads + key chunks) alternates rings so no head's
            # weights queue behind a bulky key-chunk transfer it doesn't need.
            nc.scalar.dma_start(wk[:, 0, :, :], r_wk[:, 0, :, :])
            nc.scalar.dma_start(bk[:], d_bk[:])
            nc.sync.dma_start(kuF[:], r_kuF[:])
            # key chunks split across sync + gpsimd rings; wk heads on the
            # scalar ring so no head's weights queue behind a bulk transfer
            nc.sync.dma_start(kuR[:, 0, :, 0:256], r_kuR[:, 0, :, 0:256])
            nc.gpsimd.dma_start(kuR[:, 0, :, 256:512], r_kuR[:, 0, :, 256:512])
            nc.scalar.dma_start(wk[:, 1, :, :], r_wk[:, 1, :, :])
            for c in range(1, NKR):
                nc.sync.dma_start(kuR[:, c, :, 0:256], r_kuR[:, c, :, 0:256])
                nc.gpsimd.dma_start(
                    kuR[:, c, :, 256:512], r_kuR[:, c, :, 256:512]
                )
            nc.scalar.dma_start(wk[:, 2, :, :], r_wk[:, 2, :, :])
            nc.scalar.dma_start(wk[:, 3, :, :], r_wk[:, 3, :, :])
            nc.scalar.dma_start(bq[:], d_bq[:])
            nc.scalar.dma_start(bo[:], d_bo[:])
            nc.scalar.dma_start(padb[:], d_padb[:])
            nc.scalar.dma_start(sel[:], d_sel[:])
            # vproj: vuT + own-slot wv
            nc.sync.dma_start(vuT[:, 0:4, :], r_vuT[:, 0:4, :])
            nc.scalar.dma_start(vuT[:, 4:8, :], r_vuT[:, 4:8, :])
            nc.sync.dma_start(wv[:], r_wv[:])
            # qproj: qT + wq (slot halves)
            nc.gpsimd.dma_start(qT[:, 0:4, :], r_qT[:, 0:4, :])
            nc.gpsimd.dma_start(qT[:, 4:8, :], r_qT[:, 4:8, :])
            nc.scalar.dma_start(wq[:, :, 0:512], r_wq[:, :, 0:512])
            nc.scalar.dma_start(wq[:, :, 512:1024], r_wq[:, :, 512:1024])
            # attention-phase loads: SP ring only (ACT queue = exp stream)
            nc.sync.dma_start(wo[:, 0:4, :], r_wo[:, 0:4, :])
            nc.sync.dma_start(wo[:, 4:8, :], r_wo[:, 4:8, :])

            def kproj_chunk(n, c):
                o, w = kchunks[c]
                src = kuF[:, :, :] if c == 0 else kuR[:, c - 1, :, :]
                pk = ps_proj.tile([128, 512], f32, tag="proj")
                for ht in range(HT):
                    nc.tensor.matmul(
                        pk[:, :w],
                        wk[:, n, ht, :],
                        src[:, ht, 0:w],
                        start=(ht == 0),
                        stop=(ht == HT - 1),
                    )
                nc.vector.tensor_scalar_add(
                    ksb[:, n, o : o + w], pk[:, :w], bk[:, n : n + 1]
                )

            def kproj(n):
                for c in range(len(kchunks)):
                    kproj_chunk(n, c)

            def vproj_tile(kt):
                o, klen = ktiles[kt]
                pv = ps_proj.tile([128, 512], f32, tag="proj")
                for ht in range(HT):
                    nc.tensor.matmul(
                        pv[:klen],
                        vuT[:, ht, o : o + klen],
                        wv[:, ht, :],
                        start=(ht == 0),
                        stop=(ht == HT - 1),
                    )
                nc.vector.tensor_copy(vg[:klen, 0, kt, :], pv[:klen])

            def vproj():
                for kt in range(NKT):
                    vproj_tile(kt)

            def qproj(n):
                for qc in range(QC):
                    pq = ps_proj.tile([128, 512], f32, tag="proj")
                    for ht in range(HT):
                        nc.tensor.matmul(
                            pq[:],
                            wq[:, ht, n * 128 : (n + 1) * 128],
                            qT[:, ht, qc * 512 : (qc + 1) * 512],
                            start=(ht == 0),
                            stop=(ht == HT - 1),
                        )
                    nc.vector.tensor_scalar_add(
                        qsb[:, n, qc * 512 : (qc + 1) * 512], pq[:], bq[:, n : n + 1]
                    )

            def attn_chunk(n, qc):
                """Scores + exp + PV for one (head, 512-query) chunk. The
                softmax denominator accumulates as a DVE bf16 chain over the
                exp tiles; its single matmul is deferred one chunk so the PE
                never waits on the chain."""
                qsl = slice(qc * 512, (qc + 1) * 512)
                chb = bcp.tile([128, 512], bf16, tag="chb")
                nc.gpsimd.dma_start(
                    chb[:],
                    d_chb[
                        0:1, n * Sq + qc * 512 : n * Sq + (qc + 1) * 512
                    ].to_broadcast([128, 512]),
                )
                ppv = ps_pv.tile([128, 512], f32)
                esum = esp.tile([128, 512], bf16)
                e0 = None
                k0 = 0
                for kt, (ko, klen) in enumerate(ktiles):
                    ps = ps_proj.tile([128, 512], f32, tag="proj")
                    nc.tensor.matmul(
                        ps[:klen],
                        ksb[:, n, ko : ko + klen],
                        qsb[:, n, qsl],
                        start=True,
                        stop=True,
                    )
                    e = expp.tile([128, 512], bf16)
                    nc.scalar.activation(
                        out=e[:klen],
                        in_=ps[:klen],
                        func=mybir.ActivationFunctionType.Exp,
                        bias=padb[:klen, kt : kt + 1],
                        scale=SCALE,
                    )
                    nc.tensor.matmul(
                        ppv[:],
                        vg[:klen, n // 4, kt, n % 4 * 128 : (n % 4 + 1) * 128],
                        e[:klen],
                        start=(kt == 0),
                        stop=(kt == NKT - 1),
                    )
                    if kt == 0:
                        e0, k0 = e, klen
                    elif kt == 1:
                        nc.vector.tensor_add(esum[:klen], e0[:klen], e[:klen])
                        if klen < k0:
                            nc.vector.tensor_copy(esum[klen:k0], e0[klen:k0])
                    else:
                        nc.vector.tensor_add(esum[:klen], esum[:klen], e[:klen])
                if NKT == 1:
                    nc.vector.tensor_copy(esum[:k0], e0[:k0])
                return (n, qsl, ppv, esum, chb)

            KMAX = min(128, UP)

            def attn_finish(st):
                n, qsl, ppv, esum, chb = st
                pd = ps_d.tile([128, 512], f32, tag="pd")
                nc.tensor.matmul(
                    pd[:], ones_mat[:KMAX], esum[:KMAX], start=True, stop=True
                )
                # mask-branch rows get denom += 1e30: output underflows to 0
                pda = scp.tile([128, 512], f32, tag="pda")
                nc.vector.tensor_add(pda[:], pd[:], chb[:])
                rec = scp.tile([128, 512], f32, tag="rec")
                nc.vector.reciprocal_approx_fast(rec[:], pda[:])
                nc.vector.tensor_mul(out_all[:, n, qsl], ppv[:], rec[:])

            r_yT = d_yT[:].rearrange("(t p) s -> t p s", p=128)

            def outproj_piece(qc, ht):
                py = ps_proj.tile([128, 512], f32, tag="proj")
                for n in range(NH):
                    nc.tensor.matmul(
                        py[:],
                        wo[:, n, ht * 128 : (ht + 1) * 128],
                        out_all[:, n, qc * 512 : (qc + 1) * 512],
                        start=(n == 0),
                        stop=(n == NH - 1),
                    )
                yt = ytp.tile([128, 512], f32)
                q0 = qc * 512
                if qc == QC - 1:
                    # tail: exp stream is done, split bias-add + store in
                    # halves across both HWDGE rings so the final piece's
                    # store begins as soon as its first half is ready
                    nc.vector.tensor_scalar_add(
                        yt[:, 0:256], py[:, 0:256], bo[:, ht : ht + 1]
                    )
                    nc.sync.dma_start(r_yT[ht, :, q0 : q0 + 256], yt[:, 0:256])
                    nc.vector.tensor_scalar_add(
                        yt[:, 256:512], py[:, 256:512], bo[:, ht : ht + 1]
                    )
                    nc.scalar.dma_start(
                        r_yT[ht, :, q0 + 256 : q0 + 512], yt[:, 256:512]
                    )
                else:
                    nc.vector.tensor_scalar_add(
                        yt[:], py[:], bo[:, ht : ht + 1]
                    )
                    nc.sync.dma_start(r_yT[ht, :, q0 : q0 + 512], yt[:])

            # ---- interleaved schedule with pairwise K/V-projection dedup --
            # Each core projects only its OWN half of K (4 head-slots) and V
            # (its 512 dk columns); the halves are exchanged between the two
            # cores sharing a batch via a 2-rank AllGather (~60us control
            # latency, hidden under own-slot attention). Head-slot order is
            # per-core (host permutes the weights): slots 0-3 are always the
            # locally-projected heads, so attention on them starts without
            # waiting for the exchange. Slots 4-7 are selected out of the
            # gathered buffer with a per-core 0/1 coefficient pair (`sel`),
            # because the AllGather output is rank-ordered while "peer" is
            # rank-dependent -- a compile-time slice can't express it under
            # SPMD, but b0*c0 + b1*c1 with host-set c can.
            pending = []

            def emit_chunk(n, qc):
                st = attn_chunk(n, qc)
                pending.append(st)
                if len(pending) > 2:
                    attn_finish(pending.pop(0))

            rg = [[2 * i, 2 * i + 1] for i in range(NCORES // 2)]

            for n in range(4):
                kproj(n)
            nc.scalar.dma_start(kown[:], ksb[:, 0:4, :])
            nc.gpsimd.collective_compute(
                "AllGather",
                mybir.AluOpType.bypass,
                replica_groups=rg,
                ins=[kown[:].opt()],
                outs=[kall[:].opt()],
            )
            vproj()
            nc.scalar.dma_start(vown[:], vg[:, 0, :, :])
            nc.gpsimd.collective_compute(
                "AllGather",
                mybir.AluOpType.bypass,
                replica_groups=rg,
                ins=[vown[:].opt()],
                outs=[vall[:].opt()],
            )
            for n in range(4):
                qproj(n)
            # own-slot attention (both query chunks) while the exchange is
            # in flight; qproj of the peer slots interleaved
            emit_chunk(0, 0)
            emit_chunk(0, 1)
            qproj(4)
            emit_chunk(1, 0)
            emit_chunk(1, 1)
            qproj(5)
            # select the peer half out of the gathered buffers:
            #   ksb[:,4:8] = kall[0]*sel0 + kall[1]*sel1   (vg likewise)
            stk0 = stgp.tile([128, 4, UP], bf16, tag="stg")
            stk1 = stgp.tile([128, 4, UP], bf16, tag="stg")
            nc.sync.dma_start(stk0[:], kall[0, :, :, :])
            nc.sync.dma_start(stk1[:], kall[1, :, :, :])
            nc.vector.tensor_scalar_mul(ksb[:, 4:8, :], stk0[:], sel[:, 0:1])
            nc.vector.tensor_scalar_mul(stk0[:], stk1[:], sel[:, 1:2])
            nc.vector.tensor_add(ksb[:, 4:8, :], ksb[:, 4:8, :], stk0[:])
            emit_chunk(2, 0)
            emit_chunk(2, 1)
            qproj(6)
            stv0 = stgp.tile([128, NKT, 512], bf16, tag="stg")
            stv1 = stgp.tile([128, NKT, 512], bf16, tag="stg")
            nc.sync.dma_start(stv0[:], vall[0, :, :, :])
            nc.sync.dma_start(stv1[:], vall[1, :, :, :])
            nc.vector.tensor_scalar_mul(vg[:, 1, :, :], stv0[:], sel[:, 0:1])
            nc.vector.tensor_scalar_mul(stv0[:], stv1[:], sel[:, 1:2])
            nc.vector.tensor_add(vg[:, 1, :, :], vg[:, 1, :, :], stv0[:])
            emit_chunk(3, 0)
            emit_chunk(3, 1)
            qproj(7)
            # peer-slot attention; qc0 output pieces interleave once all
            # slots' qc0 chunks are done
            for n in range(4, NH):
                emit_chunk(n, 0)
            for j, n in enumerate(range(4, NH)):
                emit_chunk(n, 1)
                outproj_piece(0, 2 * j)
                outproj_piece(0, 2 * j + 1)
            while pending:
                attn_finish(pending.pop(0))
            for ht in range(HT):
                outproj_piece(1, ht)

    nc.compile()
    return nc


def _prepare(query, key, value, key_padding_mask, Wq, bq, Wk, bk, Wv, bv, Wo, bo):
    """Host-side prep: mask constants (fp64), gather/transpose, per-core maps."""
    mask = np.asarray(key_padding_mask)
    q64 = np.asarray(query, np.float64)
    Wq64 = np.asarray(Wq, np.float64)
    Wk64 = np.asarray(Wk, np.float64)
    Wv64 = np.asarray(Wv, np.float64)
    Wo64 = np.asarray(Wo, np.float64)

    # shared projected row of all masked keys, per head
    kmask = NEG * Wk64.sum(axis=1) + np.asarray(bk, np.float64)  # [NH, DK]

    # z sign per (s, b, n):  z = q . (Wq[n] @ kmask[n]) + bq[n].kmask[n]
    wz = np.einsum("nhd,nd->hn", Wq64, kmask)  # [H, NH]
    cz = np.einsum("nd,nd->n", np.asarray(bq, np.float64), kmask)  # [NH]
    z = q64.reshape(S * B, H) @ wz + cz  # [S*B, NH]
    choose = (z > 0).reshape(S, B, NH)

    # mask-branch output: mean of (unmasked-data) V over masked key positions
    v64 = np.asarray(value, np.float64)  # [S, B, H]
    vbar_feat = np.stack(
        [
            v64[mask[b], b, :].mean(axis=0)
            if mask[b].any()
            else np.zeros(H)
            for b in range(B)
        ]
    )  # [B, H]
    for b in range(B):
        if not mask[b].any():
            choose[:, b, :] = False  # no masked keys -> no mask branch
        elif mask[b].all():
            # all keys masked: identical scores -> uniform softmax -> Vbar
            choose[:, b, :] = True
    # bv is folded into bo on the device, so the host correction uses vbar
    # WITHOUT bv (the device adds bv@Wo to every row via the output bias).
    vbar = np.einsum("bh,nhd->bnd", vbar_feat, Wv64)  # [B, NH, DK]
    ubar = np.einsum(
        "bnd,ndh->bnh", vbar, Wo64.reshape(NH, DK, H)
    )  # [B, NH, H]

    # correction added on host for mask-branch rows
    ycorr = np.einsum("sbn,bnh->sbh", choose.astype(np.float64), ubar)

    # gather unmasked keys per batch
    idx = [np.nonzero(~mask[b])[0] for b in range(B)]
    umax = max(max(len(i) for i in idx), 1)
    UP = umax
    NKT = (UP + 127) // 128

    # Per-parity head-slot permutation: odd cores process heads
    # [4,5,6,7,0,1,2,3] so that "slots 0-3" are always the locally-projected
    # half of the K/V exchange. The final y sums over all heads, so the
    # permutation does not change the output.
    perms = [np.arange(NH), np.array([4, 5, 6, 7, 0, 1, 2, 3])]
    Wq_p, Wk_p, Wv_p, Wo_p, bq_p, bk_p = [], [], [], [], [], []
    for pi in perms:
        Wq_p.append(
            np.ascontiguousarray(
                np.asarray(Wq)[pi].transpose(1, 0, 2).reshape(H, NHDK)
            ).astype(npbf16)
        )
        # own half only (slots 0-3), head-major partition-major layout
        Wk_p.append(
            np.ascontiguousarray(
                np.asarray(Wk)[pi[:4]]
                .reshape(4, HT, 128, DK)
                .transpose(0, 2, 1, 3)
            ).reshape(4 * 128, HT * DK).astype(npbf16)
        )
        Wv_p.append(
            np.ascontiguousarray(
                np.asarray(Wv)[pi[:4]].transpose(1, 0, 2).reshape(H, 512)
            ).astype(npbf16)
        )
        Wo_p.append(
            np.ascontiguousarray(
                np.asarray(Wo, np.float32).reshape(NH, DK, H)[pi]
            ).reshape(NHDK, H).astype(npbf16)
        )
        bq_p.append(
            np.ascontiguousarray(np.asarray(bq, np.float32)[pi].T)
        )
        bk_p.append(
            np.ascontiguousarray(np.asarray(bk, np.float32)[pi[:4]].T)
        )
    sel_p = [
        np.broadcast_to(np.array(s, np.float32), (128, 2)).copy()
        for s in ([0.0, 1.0], [1.0, 0.0])
    ]
    # fold bv into the output bias: y += bv_flat @ Wo (exact: softmax
    # weights sum to 1, so out = PV/d + bv before the output projection)
    bo_eff = np.asarray(bo, np.float64) + np.asarray(bv, np.float64).reshape(
        NHDK
    ) @ Wo64
    bo_d = np.ascontiguousarray(
        bo_eff.astype(np.float32).reshape(HT, 128).T
    )  # [128, HT]

    Sq = S // 2
    # kproj chunk grid (must match build_program)
    K0 = min(256, UP)
    NKR = 0
    o = K0
    while o < UP:
        NKR += 1
        o += 512
    in_maps = []
    for core in range(NCORES):
        b, half = divmod(core, 2)
        qo = half * Sq
        ii = idx[b]
        u = len(ii)
        kuT = np.zeros((H, UP), npbf16)
        kuT[:, :u] = np.asarray(key[ii, b, :], np.float32).T.astype(npbf16)
        # key-chunk-major repack: kuF [128, HT*K0]; kuR [128, NKR*HT*512]
        ku3 = kuT.reshape(HT, 128, UP)  # [t, p, u]
        kuF = np.ascontiguousarray(
            ku3[:, :, 0:K0].transpose(1, 0, 2).reshape(128, HT * K0)
        )
        kuR4 = np.zeros((128, max(NKR, 1), HT, 512), npbf16)
        for c in range(NKR):
            o0 = K0 + c * 512
            w = min(512, UP - o0)
            kuR4[:, c, :, 0:w] = ku3[:, :, o0 : o0 + w].transpose(1, 0, 2)
        kuR = np.ascontiguousarray(kuR4.reshape(128, max(NKR, 1) * HT * 512))
        vuT = np.zeros((H, UP), npbf16)
        vuT[:, :u] = np.asarray(value[ii, b, :], np.float32).T.astype(npbf16)
        qT = np.ascontiguousarray(
            np.asarray(query[qo : qo + Sq, b, :], np.float32).T
        ).astype(npbf16)
        padb = np.zeros((128, NKT), np.float32)
        flat = np.arange(NKT * 128).reshape(NKT, 128).T  # [128, NKT] key index
        padb[flat >= max(u, 1)] = -30000.0  # keep >=1 live key (denom > 0)
        pi = perms[core % 2]
        chb = np.ascontiguousarray(
            (
                choose[qo : qo + Sq, b, :][:, pi].T.astype(np.float32) * 1.0e30
            ).astype(npbf16)
        ).reshape(1, NH * Sq)
        par = core % 2
        in_maps.append(
            {
                "qT": qT,
                "kuF": kuF,
                "kuR": kuR,
                "vuT": vuT,
                "wq": Wq_p[par],
                "wk": Wk_p[par],
                "wv": Wv_p[par],
                "wo": Wo_p[par],
                "bq": bq_p[par],
                "bk": bk_p[par],
                "bo": bo_d,
                "padb": padb,
                "chb": chb,
                "sel": sel_p[par],
            }
        )
    return in_maps, ycorr, Sq, UP


def run(inputs: dict, trace: bool = False):
    in_maps, ycorr, Sq, UP = _prepare(**inputs)
    key_ = (Sq, UP)
    if key_ not in _PROG_CACHE:
        _PROG_CACHE[key_] = build_program(Sq, UP)
    nc = _PROG_CACHE[key_]
    res = run_bass_kernel_spmd(nc, in_maps, list(range(NCORES)), trace=trace)
    y = np.empty((S, B, H), np.float32)
    for core in range(NCORES):
        b, half = divmod(core, 2)
        qo = half * Sq
        y[qo : qo + Sq, b, :] = res.results[core]["yT"].T
    y += ycorr.astype(np.float32)
    return y, res


def kernel(**inputs) -> np.ndarray:
    y, _ = run(inputs, trace=False)
    return y

